# revision 26
# baseline (speedup 1.0000x reference)
"""Trainium2 Bass kernel for nn_MultiHeadDotProductAttention_76725295776285.

Full multi-head attention (B=2, Q=K=4096, F=512, H=8, D=64) on 8 NeuronCores.

Sharding: core c handles batch b = c//4 and q-rows [(c%4)*1024, (c%4+1)*1024).
Each core computes all 8 heads for its q-slice, so the output projection sums
over heads locally and no collective is needed.

Schedule (v2): the ScalarE exp stream (997ns per [128,1024] slab, 256 slabs)
is the floor; everything else is arranged so the PE never starves it:
  - SE (S^T+exp) order: per head-pair window, kb-major across (qb0, qb1);
    AV consumes iteration-major (PSUM accumulator lifetime); pT pool holds
    the lag (per-slab consumption guard).
  - K/Q projections for head-pair h>0 are deferred into window h-1 with the
    xkv blocks re-DMAed (weights stay resident), so the ramp only pays
    kproj/qproj hdc0 + all of vproj.
  - PE emission batches two S^T pairs then four AV matmuls to halve the PE
    tile-config switch cost (~103ns per switch).
  - exp ACT table preloaded with a dummy activation at t=0.
  - output DMAs spread across queues; final iteration tail pipelined per
    128-row q chunk.
"""

import os
import sys

for _p in ("/opt/trn_rl_repo", "/root/.axon_site/_ro/trn_rl_repo"):
    if os.path.isdir(_p) and _p not in sys.path:
        sys.path.append(_p)

import numpy as np

import concourse.bacc as bacc
import concourse.tile as tile
from concourse import mybir
from concourse.bass_utils import run_bass_kernel_spmd

B, Q, K, F, H, D = 2, 4096, 4096, 512, 8, 64
HD = H * D            # 512
NCORES = 8
QSH = Q // 4          # 1024 q rows per core
FC = F // 128         # 4 F chunks
HDC = HD // 128       # 4 hd chunks
NKB = K // 512        # 8 k blocks (DMA/projection granularity)
NKC = K // 128        # 32 k chunks (attention granularity)
NQB = QSH // 512      # 2 q blocks per core
NHP = H // 2          # 4 head pairs

ROLL = 14             # rolling pT pool depth
CACHED = 23           # long-lived pT slabs (window-0 qb1, kc<CACHED)
AV_MARGIN = 6         # emit AV only when its exp is >= this many slabs old

f32 = mybir.dt.float32
f32r = mybir.dt.float32r
f16 = mybir.dt.float16
bf16 = mybir.dt.bfloat16

_cache = {}
last_result = None  # BassKernelResults of the most recent run (for profiling)


def _build_program():
    nc = bacc.Bacc("TRN2", target_bir_lowering=False, debug=False,
                   num_devices=NCORES)

    xqT = nc.dram_tensor("xqT", [F, QSH], f16, kind="ExternalInput")
    xkvT = nc.dram_tensor("xkvT", [F, K], f16, kind="ExternalInput")
    wq = nc.dram_tensor("wq", [F, HD], f16, kind="ExternalInput")
    wk = nc.dram_tensor("wk", [F, HD], f16, kind="ExternalInput")
    wv = nc.dram_tensor("wv", [F, HD], f16, kind="ExternalInput")
    wo = nc.dram_tensor("wo", [HD, F], bf16, kind="ExternalInput")
    ones64 = nc.dram_tensor("ones64", [1, 64], f32r, kind="ExternalInput")
    out = nc.dram_tensor("out", [QSH, F], f32, kind="ExternalOutput")

    # partition-major views: row index (c*128 + p) -> [p, c, :]
    xqT_r = xqT.rearrange("(c p) q -> p c q", p=128)
    xkvT_r = xkvT.rearrange("(c p) k -> p c k", p=128)
    wq_r = wq.rearrange("(c p) n -> p c n", p=128)
    wk_r = wk.rearrange("(c p) n -> p c n", p=128)
    wv_r = wv.rearrange("(c p) n -> p c n", p=128)
    wo_r = wo.rearrange("(c p) n -> p c n", p=128)

    dma_queues = [nc.sync, nc.gpsimd]

    with tile.TileContext(nc) as tc:
        with (
            tc.tile_pool(name="persist", bufs=1) as persist,
            tc.tile_pool(name="stream", bufs=3) as stream,
            tc.tile_pool(name="ptp", bufs=ROLL) as ptp,
            tc.tile_pool(name="small", bufs=4) as small,
            tc.tile_pool(name="psum", bufs=2, space="PSUM") as psum,
        ):
            # ---- persistent SBUF tensors ----
            qT_sb = persist.tile([128, HDC, QSH], f16, tag="qT")
            kT_sb = [persist.tile([128, HDC, 512], f16, tag=f"kT{kb}",
                                  name=f"kT{kb}")
                     for kb in range(NKB)]
            v_sb = [persist.tile([128, H, 65], bf16, tag=f"v{kc}",
                                 name=f"v{kc}")
                    for kc in range(NKC)]
            outT_sb = persist.tile([128, HDC, QSH], bf16, tag="outT")
            wk_sb = persist.tile([128, FC, HD], f16, tag="wk")
            wv_sb = persist.tile([128, FC, HD], f16, tag="wv")
            wq_sb = persist.tile([128, FC, HD], f16, tag="wq")
            wo_bf = persist.tile([128, HDC, F], bf16, tag="wo_bf")
            ones_sb = persist.tile([65, 64], f32r, tag="ones")
            scr = persist.tile([128, 8], f32, tag="scr")

            # preload the exp ACT table set off the critical path
            nc.vector.memset(scr[:], 0.0)
            nc.scalar.activation(out=scr[:], in_=scr[:],
                                 func=mybir.ActivationFunctionType.Exp)

            # ---- input DMAs ----
            xq_blk = [None, None]
            xkv_blk = [None] * NKB

            xq_blk[0] = stream.tile([128, FC, 512], f16, tag="xblk",
                                    name="xqb0")
            xkv_blk[0] = stream.tile([128, FC, 512], f16, tag="xblk",
                                     name="xkvb0")
            for fc in range(FC):
                nc.sync.dma_start(out=wq_sb[:, fc, :], in_=wq_r[:, fc, :])
                nc.sync.dma_start(out=xq_blk[0][:, fc, :],
                                  in_=xqT_r[:, fc, 0:512])
            for fc in range(FC):
                nc.scalar.dma_start(out=wk_sb[:, fc, :], in_=wk_r[:, fc, :])
                nc.scalar.dma_start(out=xkv_blk[0][:, fc, :],
                                  in_=xkvT_r[:, fc, 0:512])
            nc.scalar.dma_start(out=wv_sb[:], in_=wv_r[:])
            xq_blk[1] = stream.tile([128, FC, 512], f16, tag="xblk",
                                    name="xqb1")
            nc.sync.dma_start(out=xq_blk[1][:], in_=xqT_r[:, :, 512:1024])
            nc.gpsimd.dma_start(out=wo_bf[:], in_=wo_r[:])
            nc.sync.dma_start(out=ones_sb[64:65, :], in_=ones64[:])

            # ---- projection emitters ----
            def emit_qproj(blk, qb, hdcs):
                for hdc in hdcs:
                    acc = psum.tile([128, 512], f32, tag="proj",
                                    name=f"qacc{qb}_{hdc}")
                    for fc in range(FC):
                        nc.tensor.matmul(
                            acc[:],
                            wq_sb[:, fc, hdc * 128:(hdc + 1) * 128],
                            blk[:, fc, :],
                            start=(fc == 0), stop=(fc == FC - 1),
                        )
                    nc.vector.tensor_copy(
                        qT_sb[:, hdc, qb * 512:(qb + 1) * 512], acc[:])

            def emit_kproj(blk, kb, hdcs):
                for hdc in hdcs:
                    acc = psum.tile([128, 512], f32, tag="proj",
                                    name=f"kacc{kb}_{hdc}")
                    for fc in range(FC):
                        nc.tensor.matmul(
                            acc[:],
                            wk_sb[:, fc, hdc * 128:(hdc + 1) * 128],
                            blk[:, fc, :],
                            start=(fc == 0), stop=(fc == FC - 1),
                        )
                    nc.vector.tensor_copy(kT_sb[kb][:, hdc, :], acc[:])

            def emit_vproj(kb):
                blk = xkv_blk[kb]
                for ks in range(4):
                    kc = kb * 4 + ks
                    acc = psum.tile([128, 512], f32, tag="proj",
                                    name=f"vacc{kb}_{ks}")
                    for fc in range(FC):
                        nc.tensor.matmul(
                            acc[:],
                            blk[:, fc, ks * 128:(ks + 1) * 128],
                            wv_sb[:, fc, :],
                            start=(fc == 0), stop=(fc == FC - 1),
                        )
                    nc.vector.tensor_copy(
                        v_sb[kc][:, :, 0:64],
                        acc.rearrange("p (h d) -> p h d", h=H))
                    nc.vector.memset(v_sb[kc][:, :, 64:65], 1.0)

            # ---- attention emitters ----
            def emit_st_exp(hp, qb, kc, cached=False):
                kb, ks = kc // 4, kc % 4
                st = psum.tile([128, 2, 512], f32, tag="st",
                               name=f"st{hp}_{qb}_{kc}")
                for hi in range(2):
                    nc.tensor.matmul(
                        st[:, hi, :],
                        kT_sb[kb][hi * 64:(hi + 1) * 64, hp,
                                  ks * 128:(ks + 1) * 128],
                        qT_sb[hi * 64:(hi + 1) * 64, hp,
                              qb * 512:(qb + 1) * 512],
                        start=True, stop=True,
                        tile_position=(hi * 64, 0),
                    )
                pT = ptp.tile([128, 2, 512], bf16,
                              tag="pTc" if cached else "pT",
                              bufs=CACHED if cached else ROLL,
                              name=f"pT{hp}_{qb}_{kc}")
                nc.scalar.activation(
                    out=pT[:], in_=st[:],
                    func=mybir.ActivationFunctionType.Exp)
                return pT

            def emit_av(hp, qb, kc, av, pT):
                for hi in range(2):
                    nc.tensor.matmul(
                        av[hi][:],
                        v_sb[kc][:, hp * 2 + hi, :],
                        pT[:, hi, :],
                        start=(kc == 0), stop=(kc == NKC - 1),
                    )

            def alloc_av(hp, qb):
                return [psum.tile([65, 512], f32, tag="av",
                                  name=f"av{hp}_{qb}_{hi}")
                        for hi in range(2)]

            def emit_u_copies(hp, qb, av):
                us = []
                for hi in range(2):
                    u = small.tile([65, 512], f32r, tag="uav", bufs=3,
                                   name=f"u{hp}_{qb}_{hi}")
                    with nc.allow_low_precision(
                            reason="f32r attn output staging"):
                        nc.vector.tensor_copy(u[:], av[hi][:])
                    us.append(u)
                return (hp, qb, us)

            def emit_outproj(qb):
                for j in range(4):
                    qc = qb * 4 + j
                    acc = psum.tile([128, 512], f32, tag="proj",
                                    name=f"oacc{qc}")
                    for hdc in range(HDC):
                        nc.tensor.matmul(
                            acc[:],
                            outT_sb[:, hdc, qc * 128:(qc + 1) * 128],
                            wo_bf[:, hdc, :],
                            start=(hdc == 0), stop=(hdc == HDC - 1),
                        )
                    ostage = small.tile([128, 512], f32, tag="ost", bufs=2,
                                        name=f"ost{qc}")
                    nc.vector.tensor_copy(ostage[:], acc[:])
                    dma_queues[j % len(dma_queues)].dma_start(
                        out=out[qc * 128:(qc + 1) * 128, :], in_=ostage[:])

            def bcast_den(u, name):
                bc = psum.tile([64, 512], f32, tag="proj", name=name)
                nc.tensor.matmul(bc[:], ones_sb[64:65, :], u[64:65, :],
                                 start=True, stop=True)
                return bc

            def emit_tail_rest(p):
                hp, qb, us = p
                bcs = [bcast_den(us[hi], f"bc{hp}_{qb}_{hi}")
                       for hi in range(2)]
                for hi in range(2):
                    rbc = small.tile([64, 512], f32, tag="rbc", bufs=2,
                                     name=f"rbc{hp}_{qb}_{hi}")
                    nc.vector.reciprocal(rbc[:], bcs[hi][:])
                    nc.vector.tensor_mul(
                        outT_sb[hi * 64:(hi + 1) * 64, hp,
                                qb * 512:(qb + 1) * 512],
                        us[hi][0:64, :], rbc[:])

            # ---- schedule ----
            # iterations i = 2*hp + qb; SE order: per hp window, kb-major
            # across (qb0, qb1); AV order: iteration-major.
            iters = [(hp, qb) for hp in range(NHP) for qb in range(NQB)]
            NIT = len(iters)

            # SE order: window 0 (hp0) kb-major across (qb0, qb1) so the
            # per-kb projection work is paced at 8 slabs/kb; windows 1-3
            # iteration-major (kproj for hp emitted during window hp-1).
            se_list = []   # (iter_idx, kc)
            for kb in range(NKB):
                for qb in range(NQB):
                    se_list.extend((qb, kb * 4 + ks) for ks in range(4))
            for i in range(2, NIT):
                se_list.extend((i, kc) for kc in range(NKC))
            av_list = [(i, kc) for i in range(NIT) for kc in range(NKC)]

            # proj deferral schedule: (due_se_pos, kind, args)
            # deadline for kproj(kb, hp): se position 64*hp + 4*kb.
            XB2 = 2   # xblk2 re-DMA buffer depth
            ditems = []   # (proj_due, kind, args)
            for hp in range(1, NHP):
                if hp == 1:
                    kdue = [58 + 4 * kb for kb in range(NKB)]
                else:
                    kdue = [64 * (hp - 1) + 26 + 8 * kb for kb in range(NKB)]
                for kb in range(NKB):
                    ditems.append((kdue[kb], "kproj", (kb, hp)))
                q0 = 64 * hp - 10 if hp > 1 else 56
                q1 = 64 * hp + 16 if hp > 1 else 80
                ditems.append((q0, "qproj", (0, hp)))
                ditems.append((q1, "qproj", (1, hp)))
            ditems.sort(key=lambda x: x[0])
            # assign DMA dues honoring the xblk2 FIFO reuse constraint:
            # allocation n may only be emitted after allocation n-XB2's
            # consumer (proj) has been emitted.
            proj_sched = []
            for n, (pdue, kind, args) in enumerate(ditems):
                d = pdue - 14
                if n >= XB2:
                    d = max(d, ditems[n - XB2][0] + 1)
                proj_sched.append((min(d, pdue - 1), "dma_" + kind, args))
                proj_sched.append((pdue, kind, args))
            proj_sched.sort(key=lambda x: x[0])

            redma = {}

            def run_proj_item(kind, args):
                if kind == "dma_qproj":
                    qb, hp = args
                    blk = stream.tile([128, FC, 512], f16, tag="xblk2",
                                      bufs=XB2, name=f"xqr{hp}_{qb}")
                    nc.sync.dma_start(
                        out=blk[:],
                        in_=xqT_r[:, :, qb * 512:(qb + 1) * 512])
                    redma[("q", qb, hp)] = blk
                    return
                if kind == "qproj":
                    qb, hp = args
                    emit_qproj(redma.pop(("q", qb, hp)), qb, [hp])
                    return
                if kind == "dma_kproj":
                    kb, hp = args
                    blk = stream.tile([128, FC, 512], f16, tag="xblk2",
                                      bufs=XB2, name=f"xkvr{hp}_{kb}")
                    nc.sync.dma_start(
                        out=blk[:],
                        in_=xkvT_r[:, :, kb * 512:(kb + 1) * 512])
                    redma[("k", kb, hp)] = blk
                    return
                if kind == "kproj":
                    kb, hp = args
                    emit_kproj(redma.pop(("k", kb, hp)), kb, [hp])
                    return

            cache = {}
            state = dict(se_pos=0, av_pos=0, pending=None,
                         outproj_due=None, av_tiles=None, proj_pos=0,
                         kproj_done=set(), vproj_done=0)
            consumed = [False] * len(se_list)
            slab_of = {step: idx for idx, step in enumerate(se_list)}
            exp_pos_of = {}   # (i, kc) -> se position at emission
            roll_seq = []     # se positions that used the rolling pT tag

            def is_cached(step):
                return step[0] == 1 and step[1] < CACHED

            def emit_due_proj():
                while (state["proj_pos"] < len(proj_sched)
                       and proj_sched[state["proj_pos"]][0] <= state["se_pos"]):
                    _, kind, args = proj_sched[state["proj_pos"]]
                    run_proj_item(kind, args)
                    if kind == "kproj":
                        state["kproj_done"].add(args)
                    state["proj_pos"] += 1

            def se_ready():
                if state["se_pos"] >= len(se_list):
                    return False
                step = se_list[state["se_pos"]]
                i, kc = step
                hp = iters[i][0]
                kb = kc // 4
                if (kb, hp) not in state["kproj_done"]:
                    return False
                # rolling pT slot guard (cached slabs are never reused)
                if not is_cached(step):
                    r = len(roll_seq)
                    if r >= ROLL and not consumed[roll_seq[r - ROLL]]:
                        return False
                return True

            def pump_se(budget):
                n = 0
                while n < budget and se_ready():
                    step = se_list[state["se_pos"]]
                    i, kc = step
                    hp, qb = iters[i]
                    c = is_cached(step)
                    cache[step] = emit_st_exp(hp, qb, kc, cached=c)
                    if not c:
                        roll_seq.append(state["se_pos"])
                    exp_pos_of[step] = state["se_pos"]
                    state["se_pos"] += 1
                    n += 1

            def av_can_emit(margin=AV_MARGIN):
                if state["av_pos"] >= len(av_list):
                    return False
                step = av_list[state["av_pos"]]
                if step not in cache:
                    return False
                if step[1] // 4 >= state["vproj_done"]:
                    return False   # v for this chunk not yet projected
                return exp_pos_of[step] <= state["se_pos"] - margin

            def emit_av_step():
                i, kc = av_list[state["av_pos"]]
                hp, qb = iters[i]
                if kc == 0:
                    state["av_tiles"] = alloc_av(hp, qb)
                emit_av(hp, qb, kc, state["av_tiles"], cache.pop((i, kc)))
                consumed[slab_of[(i, kc)]] = True
                if kc == 4 and state["pending"] is not None:
                    emit_tail_rest(state["pending"])
                    state["outproj_due"] = (state["pending"][1]
                                            if state["pending"][0] == NHP - 1
                                            else None)
                    state["pending"] = None
                if kc == 16 and state["outproj_due"] is not None:
                    emit_outproj(state["outproj_due"])
                    state["outproj_due"] = None
                if kc == NKC - 1 and i < NIT - 1:
                    state["pending"] = emit_u_copies(hp, qb, state["av_tiles"])
                state["av_pos"] += 1

            # ---- ramp: hp0 window, kb-paced by DMA + kproj hdc0 + vproj ----
            emit_qproj(xq_blk[0], 0, hdcs=[0])
            for kb in range(NKB):
                if kb > 0:
                    xkv_blk[kb] = stream.tile([128, FC, 512], f16,
                                              tag="xblk", name=f"xkvb{kb}")
                    nc.scalar.dma_start(
                        out=xkv_blk[kb][:],
                        in_=xkvT_r[:, :, kb * 512:(kb + 1) * 512])
                emit_kproj(xkv_blk[kb], kb, hdcs=[0])
                state["kproj_done"].add((kb, 0))
                pump_se(2)
                emit_vproj(kb)
                state["vproj_done"] = kb + 1
                if kb == 0:
                    emit_qproj(xq_blk[1], 1, hdcs=[0])
                pump_se(4)
                emit_due_proj()
                while av_can_emit():
                    emit_av_step()
                    pump_se(2)

            # ---- steady: batched [2x ST pair | 4x AV] + deferred proj ----
            stall = 0
            while state["av_pos"] < len(av_list):
                emit_due_proj()
                before = (state["se_pos"], state["av_pos"])
                pump_se(2)
                for _ in range(2):
                    if av_can_emit():
                        emit_av_step()
                if (state["se_pos"], state["av_pos"]) == before:
                    # forced progress: relax the margin, then the guards
                    if av_can_emit(margin=0):
                        emit_av_step()
                    elif state["av_pos"] < len(av_list) \
                            and av_list[state["av_pos"]] in cache:
                        emit_av_step()
                    else:
                        pump_se(1)
                        stall += 1
                        if stall > 10000:
                            raise RuntimeError("schedule deadlock")

            # ---- final tail, pipelined per 128-wide q chunk ----
            fi = NIT - 1
            fhp, fqb = iters[fi]
            fus = emit_u_copies(fhp, fqb, state["av_tiles"])[2]
            fbcs = [bcast_den(fus[hi], f"fbc{hi}") for hi in range(2)]
            for j in range(4):
                qc = fqb * 4 + j
                js = slice(j * 128, (j + 1) * 128)
                for hi in range(2):
                    rbc = small.tile([64, 128], f32, tag="frbc",
                                     name=f"frbc{j}_{hi}")
                    nc.vector.reciprocal(rbc[:], fbcs[hi][:, js])
                    nc.vector.tensor_mul(
                        outT_sb[hi * 64:(hi + 1) * 64, fhp,
                                fqb * 512 + j * 128:fqb * 512 + (j + 1) * 128],
                        fus[hi][0:64, js], rbc[:])
                acc = psum.tile([128, 512], f32, tag="st", name=f"foacc{qc}")
                for hdc in range(HDC):
                    nc.tensor.matmul(
                        acc[:],
                        outT_sb[:, hdc, qc * 128:(qc + 1) * 128],
                        wo_bf[:, hdc, :],
                        start=(hdc == 0), stop=(hdc == HDC - 1),
                    )
                ostage = small.tile([128, 512], f32, tag="ost", bufs=2,
                                    name=f"fost{qc}")
                nc.vector.tensor_copy(ostage[:], acc[:])
                dma_queues[j % len(dma_queues)].dma_start(
                    out=out[qc * 128:(qc + 1) * 128, :], in_=ostage[:])

    nc.compile()
    return nc


def kernel(**inputs):
    global last_result
    import ml_dtypes
    inputs_q = np.asarray(inputs["inputs_q"], dtype=np.float32)
    inputs_kv = np.asarray(inputs["inputs_kv"], dtype=np.float32)
    Wq = np.asarray(inputs["Wq"], dtype=np.float32).reshape(F, HD).astype(np.float16)
    Wk = np.asarray(inputs["Wk"], dtype=np.float32).reshape(F, HD).astype(np.float16)
    Wv = np.asarray(inputs["Wv"], dtype=np.float32).reshape(F, HD).astype(np.float16)
    Wo = np.asarray(inputs["Wo"], dtype=np.float32).reshape(HD, F).astype(ml_dtypes.bfloat16)
    ones = np.ones((1, 64), dtype=np.float32)

    if "nc" not in _cache:
        _cache["nc"] = _build_program()
    nc = _cache["nc"]

    xkvT = [np.ascontiguousarray(inputs_kv[b].T).astype(np.float16) for b in range(B)]
    in_maps = []
    for c in range(NCORES):
        b, qi = c // 4, c % 4
        in_maps.append({
            "xqT": np.ascontiguousarray(
                inputs_q[b, qi * QSH:(qi + 1) * QSH, :].T).astype(np.float16),
            "xkvT": xkvT[b],
            "wq": Wq, "wk": Wk, "wv": Wv, "wo": Wo,
            "ones64": ones,
        })

    res = run_bass_kernel_spmd(nc, in_maps, core_ids=list(range(NCORES)))
    last_result = res

    out = np.empty((B, Q, F), dtype=np.float32)
    for c in range(NCORES):
        b, qi = c // 4, c % 4
        out[b, qi * QSH:(qi + 1) * QSH, :] = res.results[c]["out"]
    return out
